# revision 4
# baseline (speedup 1.0000x reference)
"""AttnBlock (GroupNorm + single-head HWxHW attention + residual) on 8 trn2 cores.

Sharding: data-parallel over (batch, query-half): core i handles batch i//2,
query columns [ (i%2)*2048, (i%2+1)*2048 ).  The input for odd cores is
column-rotated on the host so every core's queries are columns 0:2048 of its
input (softmax over keys is permutation invariant, so k/v order doesn't
matter) -- this keeps the program SPMD (one NEFF for all 8 cores).

Device algorithm (per core, C=128 channels on partitions, N=4096 spatial):
  - GroupNorm stats: per-channel bn_stats/bn_aggr, then group (16-channel)
    reduction + broadcast via tiny mask matmuls on the PE.
  - q/k projections as [C,C]x[C,n] matmuls; v is produced directly
    TRANSPOSED (vT[m,c]) by using h-chunks as the stationary operand.
  - Scores are computed transposed: sT[m-tile, n-block] = k_tile^T . q_blk.
    exp() runs on the scalar engine straight out of PSUM (no row-max
    subtraction needed: scores are O(10), fp32 exp is safe).
  - Softmax denominator: accumulated on the vector engine (SBUF adds),
    partition-reduced with a ones-vector matmul, reciprocal on DVE,
    broadcast back across partitions by GPSIMD.
  - PV: num[c, n] += vT_tile^T . pT_tile accumulated over 32 m-tiles in PSUM.
  - out = x + wp . num * (1/den) + bias  (division commutes past wp).

Host folding: gn_scale/gn_bias are folded into the q/k/v weights and biases;
the k bias is dropped entirely (additive per-query constant is softmax
invariant); the v bias is folded into the output projection bias since
softmax rows sum to 1.  All folded biases are applied as per-partition
scalar adds fused into copies.
"""

import os
import sys
import types

if "/opt/trn_rl_repo" not in sys.path:
    sys.path.insert(0, "/opt/trn_rl_repo")

import numpy as np

B, C, H, W = 4, 128, 64, 64
N = H * W              # 4096 spatial positions
NQ = N // 2            # 2048 queries per core
NB = 512               # query block (columns per psum bank)
NBLK = NQ // NB        # 4 query blocks
MT = N // 128          # 32 key tiles
GROUPS = 8
GSIZE = C // GROUPS    # 16 channels per group
EPS = 1e-6
SCALE = float(C) ** -0.5
EXP_GRP = 2            # psum banks (512-wide matmuls) per exp instruction

# Set to False to run all big matmuls in full fp32 (4x slower, exact).
MM_FAST = os.environ.get("KERNEL_MM_FAST", "1") == "1"

LAST_RESULTS = None    # BassKernelResults of the most recent kernel() call


def _install_ntff_hook():
    """antenv.axon_hooks is missing from this container; inject it so
    run_bass_kernel_spmd(trace=True) can capture NTFF profiles."""
    if "antenv.axon_hooks" in sys.modules:
        return
    mod = types.ModuleType("antenv.axon_hooks")
    holder = [None]
    mod.set_axon_ntff_profile_hook = lambda h: holder.__setitem__(0, h)
    mod.get_axon_ntff_profile_hook = lambda: holder[0]
    sys.modules["antenv.axon_hooks"] = mod
    try:
        from trn_agent_boot.trn_boot import _ntff_profile_via_ctypes

        mod.set_axon_ntff_profile_hook(
            _ntff_profile_via_ctypes("/opt/axon/libaxon_pjrt.so")
        )
    except Exception:
        pass


_NC_CACHE = {}


def _build(mm_fast: bool):
    if mm_fast in _NC_CACHE:
        return _NC_CACHE[mm_fast]

    import concourse.bacc as bacc
    import concourse.mybir as mybir
    import concourse.tile as tile

    f32 = mybir.dt.float32
    mmdt = mybir.dt.float32r if mm_fast else f32

    nc = bacc.Bacc("TRN2", target_bir_lowering=False, debug=False, num_devices=8)

    xp = nc.dram_tensor("xp", [C, N], f32, kind="ExternalInput")
    wqT_d = nc.dram_tensor("wqT", [C, C], f32, kind="ExternalInput")
    wkT_d = nc.dram_tensor("wkT", [C, C], f32, kind="ExternalInput")
    wvT_d = nc.dram_tensor("wvT", [C, C], f32, kind="ExternalInput")
    wpT_d = nc.dram_tensor("wpT", [C, C], f32, kind="ExternalInput")
    bq_d = nc.dram_tensor("bqe", [C, 1], f32, kind="ExternalInput")
    bp_d = nc.dram_tensor("bpe", [C, 1], f32, kind="ExternalInput")
    out_d = nc.dram_tensor("out", [C, NQ], f32, kind="ExternalOutput")

    # Group-mean reduction masks: gm averages 16 channels into each group row,
    # gmT broadcasts each group row back to its 16 channels.
    gm_np = np.zeros((C, GROUPS), np.float32)
    gmT_np = np.zeros((GROUPS, C), np.float32)
    for ch in range(C):
        gm_np[ch, ch // GSIZE] = 1.0 / (GSIZE * N)
        gmT_np[ch // GSIZE, ch] = 1.0
    gm_d = nc.inline_tensor(gm_np, "gmask")
    gmT_d = nc.inline_tensor(gmT_np, "gmaskT")

    Exp = mybir.ActivationFunctionType.Exp
    Sqrt = mybir.ActivationFunctionType.Sqrt
    add_op = mybir.AluOpType.add
    sub_op = mybir.AluOpType.subtract
    mult_op = mybir.AluOpType.mult

    with tile.TileContext(nc) as tc:
        with (
            tc.tile_pool(name="big", bufs=1) as big,
            tc.tile_pool(name="wgt", bufs=1) as wgt,
            tc.tile_pool(name="ptile", bufs=4) as ptile,
            tc.tile_pool(name="small", bufs=2) as small,
            tc.tile_pool(name="ostage", bufs=2) as ostage,
            tc.tile_pool(name="ps_s", bufs=2, space="PSUM") as ps_s,
            tc.tile_pool(name="ps_pv", bufs=1, space="PSUM") as ps_pv,
            tc.tile_pool(name="ps_den", bufs=1, space="PSUM") as ps_den,
            tc.tile_pool(name="ps_m", bufs=2, space="PSUM") as ps_m,
        ):
            # --- load inputs ---
            x_sb = big.tile([C, N], f32, tag="x")
            nc.sync.dma_start(out=x_sb[:], in_=xp.ap())
            w_q0 = wgt.tile([C, C], f32, tag="wq0")
            nc.sync.dma_start(out=w_q0[:], in_=wqT_d.ap())
            w_k0 = wgt.tile([C, C], f32, tag="wk0")
            nc.sync.dma_start(out=w_k0[:], in_=wkT_d.ap())
            w_v = wgt.tile([C, C], f32, tag="wv")
            nc.sync.dma_start(out=w_v[:], in_=wvT_d.ap())
            w_p0 = wgt.tile([C, C], f32, tag="wp0")
            nc.sync.dma_start(out=w_p0[:], in_=wpT_d.ap())
            if mm_fast:
                w_q = wgt.tile([C, C], mmdt, tag="wq")
                nc.vector.tensor_copy(out=w_q[:], in_=w_q0[:])
                w_k = wgt.tile([C, C], mmdt, tag="wk")
                nc.vector.tensor_copy(out=w_k[:], in_=w_k0[:])
                w_p = wgt.tile([C, C], mmdt, tag="wp")
                nc.vector.tensor_copy(out=w_p[:], in_=w_p0[:])
            else:
                w_q, w_k, w_p = w_q0, w_k0, w_p0
            bq_sb = wgt.tile([C, 1], f32, tag="bq")
            nc.sync.dma_start(out=bq_sb[:], in_=bq_d.ap())
            bp_sb = wgt.tile([C, 1], f32, tag="bp")
            nc.sync.dma_start(out=bp_sb[:], in_=bp_d.ap())
            gm_sb = wgt.tile([C, GROUPS], f32, tag="gm")
            nc.sync.dma_start(out=gm_sb[:], in_=gm_d.ap())
            gmT_sb = wgt.tile([GROUPS, C], f32, tag="gmT")
            nc.sync.dma_start(out=gmT_sb[:], in_=gmT_d.ap())
            ones0 = wgt.tile([C, 1], f32, tag="ones0")
            nc.vector.memset(ones0[:], 1.0)
            if mm_fast:
                ones_sb = wgt.tile([C, 1], mmdt, tag="ones")
                nc.vector.tensor_copy(out=ones_sb[:], in_=ones0[:])
            else:
                ones_sb = ones0
            eps_sb = wgt.tile([C, 1], f32, tag="eps")
            nc.vector.memset(eps_sb[:], EPS)

            # --- GroupNorm statistics ---
            stats = small.tile([C, 8, 6], f32, tag="stats")
            for j in range(8):
                nc.vector.bn_stats(
                    out=stats[:, j, :], in_=x_sb[:, j * 512 : (j + 1) * 512]
                )
            mv = small.tile([C, 2], f32, tag="mv")
            nc.vector.bn_aggr(out=mv[:], in_=stats[:])
            # t2 = per-channel [sum(x), sum(x^2)] scaled by 1/(16*N) via gm
            t2 = small.tile([C, 2], f32, tag="t2")
            nc.vector.tensor_scalar_mul(t2[:, 0:1], mv[:, 0:1], float(N))
            nc.vector.tensor_tensor(t2[:, 1:2], mv[:, 0:1], mv[:, 0:1], mult_op)
            nc.vector.tensor_tensor(t2[:, 1:2], t2[:, 1:2], mv[:, 1:2], add_op)
            nc.vector.tensor_scalar_mul(t2[:, 1:2], t2[:, 1:2], float(N))
            psg = ps_m.tile([GROUPS, 2], f32, tag="m")
            nc.tensor.matmul(psg[:], lhsT=gm_sb[:], rhs=t2[:], start=True, stop=True)
            g2 = small.tile([GROUPS, 2], f32, tag="g2")
            nc.vector.tensor_copy(out=g2[:], in_=psg[:])
            psb = ps_m.tile([C, 2], f32, tag="m")
            nc.tensor.matmul(psb[:], lhsT=gmT_sb[:], rhs=g2[:], start=True, stop=True)
            # mu = E[x]; var = E[x^2] - mu^2 ; rstd = 1/sqrt(var+eps)
            mu = small.tile([C, 1], f32, tag="mu")
            nc.vector.tensor_copy(out=mu[:], in_=psb[:, 0:1])
            var = small.tile([C, 1], f32, tag="var")
            nc.vector.tensor_tensor(var[:], mu[:], mu[:], mult_op)
            nc.vector.tensor_tensor(var[:], psb[:, 1:2], var[:], sub_op)
            sd = small.tile([C, 1], f32, tag="sd")
            nc.scalar.activation(out=sd[:], in_=var[:], func=Sqrt, bias=eps_sb[:])
            rstd = small.tile([C, 1], f32, tag="rstd")
            nc.vector.reciprocal(out=rstd[:], in_=sd[:])

            h_sb = big.tile([C, N], mmdt, tag="h")
            nc.vector.tensor_scalar(
                h_sb[:], x_sb[:], mu[:], rstd[:], op0=sub_op, op1=mult_op
            )

            # --- q, k, vT projections ---
            q_sb = big.tile([C, NQ], mmdt, tag="q")
            for j in range(NQ // 512):
                psq = ps_s.tile([C, EXP_GRP, 512], f32, tag="s")
                nc.tensor.matmul(
                    psq[:, 0, :],
                    lhsT=w_q[:],
                    rhs=h_sb[:, j * 512 : (j + 1) * 512],
                    start=True,
                    stop=True,
                )
                nc.vector.tensor_scalar_add(
                    q_sb[:, j * 512 : (j + 1) * 512], psq[:, 0, :], bq_sb[:]
                )
            k_sb = big.tile([C, N], mmdt, tag="k")
            for j in range(N // 512):
                psk = ps_s.tile([C, EXP_GRP, 512], f32, tag="s")
                nc.tensor.matmul(
                    psk[:, 0, :],
                    lhsT=w_k[:],
                    rhs=h_sb[:, j * 512 : (j + 1) * 512],
                    start=True,
                    stop=True,
                )
                nc.vector.tensor_copy(out=k_sb[:, j * 512 : (j + 1) * 512], in_=psk[:, 0, :])
            vT_sb = big.tile([128, MT, C], mmdt, tag="vt")
            for mi in range(MT):
                psv = ps_s.tile([C, EXP_GRP, 512], f32, tag="s")
                nc.tensor.matmul(
                    psv[:, 0, :C],
                    lhsT=h_sb[:, mi * 128 : (mi + 1) * 128].bitcast(f32),
                    rhs=w_v[:],
                    start=True,
                    stop=True,
                )
                nc.vector.tensor_copy(out=vT_sb[:, mi, :], in_=psv[:, 0, :C])

            # --- attention over query blocks ---
            for jb in range(NBLK):
                qs = q_sb[:, jb * NB : (jb + 1) * NB]
                pv = ps_pv.tile([C, NB], f32, tag="pv")
                dn = ps_den.tile([1, NB], f32, tag="dn")
                for g in range(MT // EXP_GRP):
                    ss = ps_s.tile([128, EXP_GRP, NB], f32, tag="s")
                    for u in range(EXP_GRP):
                        mi = g * EXP_GRP + u
                        nc.tensor.matmul(
                            ss[:, u, :],
                            lhsT=k_sb[:, mi * 128 : (mi + 1) * 128],
                            rhs=qs,
                            start=True,
                            stop=True,
                        )
                    pt = ptile.tile([128, EXP_GRP, NB], mmdt, tag="pt")
                    nc.scalar.activation(out=pt[:], in_=ss[:], func=Exp, scale=SCALE)
                    for u in range(EXP_GRP):
                        mi = g * EXP_GRP + u
                        nc.tensor.matmul(
                            pv[:],
                            lhsT=vT_sb[:, mi, :],
                            rhs=pt[:, u, :],
                            start=(mi == 0),
                            stop=(mi == MT - 1),
                        )
                    for u in range(EXP_GRP):
                        mi = g * EXP_GRP + u
                        nc.tensor.matmul(
                            dn[:],
                            lhsT=ones_sb[:],
                            rhs=pt[:, u, :],
                            start=(mi == 0),
                            stop=(mi == MT - 1),
                        )
                # denominator: reciprocal + partition broadcast
                dsb = small.tile([1, NB], f32, tag="dsb")
                nc.vector.tensor_copy(out=dsb[:], in_=dn[:])
                rden = small.tile([1, NB], f32, tag="rden")
                nc.vector.reciprocal(out=rden[:], in_=dsb[:])
                rb = ostage.tile([128, NB], f32, tag="rb")
                nc.gpsimd.partition_broadcast(rb[:], rden[:])
                # output projection on the un-normalized numerator
                hv = ostage.tile([C, NB], mmdt, tag="hv")
                nc.vector.tensor_copy(out=hv[:], in_=pv[:])
                pso = ps_m.tile([C, NB], f32, tag="m")
                nc.tensor.matmul(
                    pso[:], lhsT=w_p[:], rhs=hv[:], start=True, stop=True
                )
                o1 = ostage.tile([C, NB], f32, tag="o1")
                nc.vector.tensor_tensor(o1[:], pso[:], rb[:], mult_op)
                nc.vector.tensor_scalar_add(o1[:], o1[:], bp_sb[:])
                nc.vector.tensor_tensor(
                    o1[:], o1[:], x_sb[:, jb * NB : (jb + 1) * NB], add_op
                )
                nc.sync.dma_start(out=out_d[:, jb * NB : (jb + 1) * NB], in_=o1[:])

    nc.compile()
    _NC_CACHE[mm_fast] = nc
    return nc


def kernel(**inputs):
    global LAST_RESULTS
    _install_ntff_hook()
    from concourse.bass_utils import run_bass_kernel_spmd

    ins = {k: np.ascontiguousarray(np.asarray(v), dtype=np.float32) for k, v in inputs.items()}
    x = ins["x"]
    gs, gb = ins["gn_scale"], ins["gn_bias"]

    # Fold the GroupNorm affine into the q/k/v weights; pre-transpose all
    # weights into the [in_channel, out_channel] layout the PE wants.
    wq_e = ins["wq"] * gs[None, :]
    wk_e = ins["wk"] * gs[None, :]
    wv_e = ins["wv"] * gs[None, :]
    wqT = np.ascontiguousarray(wq_e.T)
    wkT = np.ascontiguousarray(wk_e.T)
    wvT = np.ascontiguousarray(wv_e.T)
    wpT = np.ascontiguousarray(ins["wp"].T)
    bq_e = (ins["bq"] + ins["wq"] @ gb).reshape(C, 1)
    bv_e = ins["bv"] + ins["wv"] @ gb
    bp_e = (ins["bp"] + ins["wp"] @ bv_e).reshape(C, 1)

    nc = _build(MM_FAST)

    in_maps = []
    for core in range(8):
        b, half = core // 2, core % 2
        xb = x[b].reshape(C, N)
        if half == 1:
            xb = np.concatenate([xb[:, NQ:], xb[:, :NQ]], axis=1)
        in_maps.append(
            {
                "xp": np.ascontiguousarray(xb),
                "wqT": wqT,
                "wkT": wkT,
                "wvT": wvT,
                "wpT": wpT,
                "bqe": bq_e,
                "bpe": bp_e,
            }
        )

    trace = os.environ.get("KERNEL_TRACE", "0") == "1"
    res = run_bass_kernel_spmd(
        nc, in_maps, core_ids=list(range(8)), trace=trace
    )
    LAST_RESULTS = res

    out = np.empty((B, C, N), np.float32)
    for core in range(8):
        b, half = core // 2, core % 2
        out[b, :, half * NQ : (half + 1) * NQ] = res.results[core]["out"]
    return out.reshape(B, C, H, W)


# revision 5
# speedup vs baseline: 1.0291x; 1.0291x over previous
"""AttnBlock (GroupNorm + single-head HWxHW attention + residual) on 8 trn2 cores.

Sharding: data-parallel over (batch, query-half): core i handles batch i//2,
query columns [ (i%2)*2048, (i%2+1)*2048 ).  The input for odd cores is
column-rotated on the host so every core's queries are columns 0:2048 of its
input (softmax over keys is permutation invariant, so k/v order doesn't
matter) -- this keeps the program SPMD (one NEFF for all 8 cores).

Device algorithm (per core, C=128 channels on partitions, N=4096 spatial):
  - GroupNorm stats: per-channel bn_stats/bn_aggr, then group (16-channel)
    reduction + broadcast via tiny mask matmuls on the PE.
  - q/k projections as [C,C]x[C,n] matmuls; v is produced directly
    TRANSPOSED (vT[m,c]) by using h-chunks as the stationary operand.
  - Scores are computed transposed: sT[m-tile, n-block] = k_tile^T . q_blk.
    exp() runs on the scalar engine straight out of PSUM (no row-max
    subtraction needed: scores are O(10), fp32 exp is safe).
  - Softmax denominator: accumulated on the vector engine (SBUF adds),
    partition-reduced with a ones-vector matmul, reciprocal on DVE,
    broadcast back across partitions by GPSIMD.
  - PV: num[c, n] += vT_tile^T . pT_tile accumulated over 32 m-tiles in PSUM.
  - out = x + wp . num * (1/den) + bias  (division commutes past wp).

Host folding: gn_scale/gn_bias are folded into the q/k/v weights and biases;
the k bias is dropped entirely (additive per-query constant is softmax
invariant); the v bias is folded into the output projection bias since
softmax rows sum to 1.  All folded biases are applied as per-partition
scalar adds fused into copies.
"""

import os
import sys
import types

if "/opt/trn_rl_repo" not in sys.path:
    sys.path.insert(0, "/opt/trn_rl_repo")

import numpy as np

B, C, H, W = 4, 128, 64, 64
N = H * W              # 4096 spatial positions
NQ = N // 2            # 2048 queries per core
NB = 512               # query block (columns per psum bank)
NBLK = NQ // NB        # 4 query blocks
MT = N // 128          # 32 key tiles
GROUPS = 8
GSIZE = C // GROUPS    # 16 channels per group
EPS = 1e-6
SCALE = float(C) ** -0.5
EXP_GRP = 2            # psum banks (512-wide matmuls) per exp instruction

# Set to False to run all big matmuls in full fp32 (4x slower, exact).
MM_FAST = os.environ.get("KERNEL_MM_FAST", "1") == "1"

LAST_RESULTS = None    # BassKernelResults of the most recent kernel() call


def _install_ntff_hook():
    """antenv.axon_hooks is missing from this container; inject it so
    run_bass_kernel_spmd(trace=True) can capture NTFF profiles."""
    if "antenv.axon_hooks" in sys.modules:
        return
    mod = types.ModuleType("antenv.axon_hooks")
    holder = [None]
    mod.set_axon_ntff_profile_hook = lambda h: holder.__setitem__(0, h)
    mod.get_axon_ntff_profile_hook = lambda: holder[0]
    sys.modules["antenv.axon_hooks"] = mod
    try:
        from trn_agent_boot.trn_boot import _ntff_profile_via_ctypes

        mod.set_axon_ntff_profile_hook(
            _ntff_profile_via_ctypes("/opt/axon/libaxon_pjrt.so")
        )
    except Exception:
        pass


_NC_CACHE = {}


def _build(mm_fast: bool):
    if mm_fast in _NC_CACHE:
        return _NC_CACHE[mm_fast]

    import concourse.bacc as bacc
    import concourse.mybir as mybir
    import concourse.tile as tile

    f32 = mybir.dt.float32
    mmdt = mybir.dt.float32r if mm_fast else f32

    nc = bacc.Bacc("TRN2", target_bir_lowering=False, debug=False, num_devices=8)

    xp = nc.dram_tensor("xp", [C, N], f32, kind="ExternalInput")
    wqT_d = nc.dram_tensor("wqT", [C, C], f32, kind="ExternalInput")
    wkT_d = nc.dram_tensor("wkT", [C, C], f32, kind="ExternalInput")
    wvT_d = nc.dram_tensor("wvT", [C, C], f32, kind="ExternalInput")
    wpT_d = nc.dram_tensor("wpT", [C, C], f32, kind="ExternalInput")
    bq_d = nc.dram_tensor("bqe", [C, 1], f32, kind="ExternalInput")
    bp_d = nc.dram_tensor("bpe", [C, 1], f32, kind="ExternalInput")
    out_d = nc.dram_tensor("out", [C, NQ], f32, kind="ExternalOutput")

    # Group-mean reduction masks: gm averages 16 channels into each group row,
    # gmT broadcasts each group row back to its 16 channels.
    gm_np = np.zeros((C, GROUPS), np.float32)
    gmT_np = np.zeros((GROUPS, C), np.float32)
    for ch in range(C):
        gm_np[ch, ch // GSIZE] = 1.0 / (GSIZE * N)
        gmT_np[ch // GSIZE, ch] = 1.0
    gm_d = nc.inline_tensor(gm_np, "gmask")
    gmT_d = nc.inline_tensor(gmT_np, "gmaskT")

    Exp = mybir.ActivationFunctionType.Exp
    Sqrt = mybir.ActivationFunctionType.Sqrt
    add_op = mybir.AluOpType.add
    sub_op = mybir.AluOpType.subtract
    mult_op = mybir.AluOpType.mult

    with tile.TileContext(nc) as tc:
        with (
            tc.tile_pool(name="big", bufs=1) as big,
            tc.tile_pool(name="wgt", bufs=1) as wgt,
            tc.tile_pool(name="ptile", bufs=6) as ptile,
            tc.tile_pool(name="small", bufs=2) as small,
            tc.tile_pool(name="ostage", bufs=2) as ostage,
            tc.tile_pool(name="ps_s", bufs=2, space="PSUM") as ps_s,
            tc.tile_pool(name="ps_pv", bufs=1, space="PSUM") as ps_pv,
            tc.tile_pool(name="ps_den", bufs=2, space="PSUM") as ps_den,
            tc.tile_pool(name="ps_m", bufs=1, space="PSUM") as ps_m,
        ):
            # --- load inputs ---
            x_sb = big.tile([C, N], f32, tag="x")
            for j in range(4):
                nc.sync.dma_start(
                    out=x_sb[:, j * 1024 : (j + 1) * 1024],
                    in_=xp.ap()[:, j * 1024 : (j + 1) * 1024],
                )
            w_q0 = wgt.tile([C, C], f32, tag="wq0")
            nc.sync.dma_start(out=w_q0[:], in_=wqT_d.ap())
            w_k0 = wgt.tile([C, C], f32, tag="wk0")
            nc.sync.dma_start(out=w_k0[:], in_=wkT_d.ap())
            w_v = wgt.tile([C, C], f32, tag="wv")
            nc.sync.dma_start(out=w_v[:], in_=wvT_d.ap())
            w_p0 = wgt.tile([C, C], f32, tag="wp0")
            nc.sync.dma_start(out=w_p0[:], in_=wpT_d.ap())
            if mm_fast:
                w_q = wgt.tile([C, C], mmdt, tag="wq")
                nc.vector.tensor_copy(out=w_q[:], in_=w_q0[:])
                w_k = wgt.tile([C, C], mmdt, tag="wk")
                nc.vector.tensor_copy(out=w_k[:], in_=w_k0[:])
                w_p = wgt.tile([C, C], mmdt, tag="wp")
                nc.vector.tensor_copy(out=w_p[:], in_=w_p0[:])
            else:
                w_q, w_k, w_p = w_q0, w_k0, w_p0
            bq_sb = wgt.tile([C, 1], f32, tag="bq")
            nc.sync.dma_start(out=bq_sb[:], in_=bq_d.ap())
            bp_sb = wgt.tile([C, 1], f32, tag="bp")
            nc.sync.dma_start(out=bp_sb[:], in_=bp_d.ap())
            gm_sb = wgt.tile([C, GROUPS], f32, tag="gm")
            nc.sync.dma_start(out=gm_sb[:], in_=gm_d.ap())
            gmT_sb = wgt.tile([GROUPS, C], f32, tag="gmT")
            nc.sync.dma_start(out=gmT_sb[:], in_=gmT_d.ap())
            ones0 = wgt.tile([C, 1], f32, tag="ones0")
            nc.vector.memset(ones0[:], 1.0)
            if mm_fast:
                ones_sb = wgt.tile([C, 1], mmdt, tag="ones")
                nc.vector.tensor_copy(out=ones_sb[:], in_=ones0[:])
            else:
                ones_sb = ones0
            eps_sb = wgt.tile([C, 1], f32, tag="eps")
            nc.vector.memset(eps_sb[:], EPS)

            # --- GroupNorm statistics ---
            stats = small.tile([C, 8, 6], f32, tag="stats")
            for j in range(8):
                nc.vector.bn_stats(
                    out=stats[:, j, :], in_=x_sb[:, j * 512 : (j + 1) * 512]
                )
            mv = small.tile([C, 2], f32, tag="mv")
            nc.vector.bn_aggr(out=mv[:], in_=stats[:])
            # t2 = per-channel [sum(x), sum(x^2)] scaled by 1/(16*N) via gm
            t2 = small.tile([C, 2], f32, tag="t2")
            nc.vector.tensor_scalar_mul(t2[:, 0:1], mv[:, 0:1], float(N))
            nc.vector.tensor_tensor(t2[:, 1:2], mv[:, 0:1], mv[:, 0:1], mult_op)
            nc.vector.tensor_tensor(t2[:, 1:2], t2[:, 1:2], mv[:, 1:2], add_op)
            nc.vector.tensor_scalar_mul(t2[:, 1:2], t2[:, 1:2], float(N))
            psg = ps_m.tile([GROUPS, 2], f32, tag="m")
            nc.tensor.matmul(psg[:], lhsT=gm_sb[:], rhs=t2[:], start=True, stop=True)
            g2 = small.tile([GROUPS, 2], f32, tag="g2")
            nc.vector.tensor_copy(out=g2[:], in_=psg[:])
            psb = ps_m.tile([C, 2], f32, tag="m")
            nc.tensor.matmul(psb[:], lhsT=gmT_sb[:], rhs=g2[:], start=True, stop=True)
            # mu = E[x]; var = E[x^2] - mu^2 ; rstd = 1/sqrt(var+eps)
            mu = small.tile([C, 1], f32, tag="mu")
            nc.vector.tensor_copy(out=mu[:], in_=psb[:, 0:1])
            var = small.tile([C, 1], f32, tag="var")
            nc.vector.tensor_tensor(var[:], mu[:], mu[:], mult_op)
            nc.vector.tensor_tensor(var[:], psb[:, 1:2], var[:], sub_op)
            sd = small.tile([C, 1], f32, tag="sd")
            nc.scalar.activation(out=sd[:], in_=var[:], func=Sqrt, bias=eps_sb[:])
            rstd = small.tile([C, 1], f32, tag="rstd")
            nc.vector.reciprocal_approx_fast(out=rstd[:], in_=sd[:])

            h_sb = big.tile([C, N], mmdt, tag="h")
            nc.vector.tensor_scalar(
                h_sb[:], x_sb[:], mu[:], rstd[:], op0=sub_op, op1=mult_op
            )

            # --- q, k, vT projections ---
            q_sb = big.tile([C, NQ], mmdt, tag="q")
            for j in range(NQ // 512):
                psq = ps_s.tile([C, EXP_GRP, 512], f32, tag="s")
                nc.tensor.matmul(
                    psq[:, 0, :],
                    lhsT=w_q[:],
                    rhs=h_sb[:, j * 512 : (j + 1) * 512],
                    start=True,
                    stop=True,
                )
                nc.vector.tensor_scalar_add(
                    q_sb[:, j * 512 : (j + 1) * 512], psq[:, 0, :], bq_sb[:]
                )
            k_sb = big.tile([C, N], mmdt, tag="k")
            for j in range(N // 512):
                psk = ps_s.tile([C, EXP_GRP, 512], f32, tag="s")
                nc.tensor.matmul(
                    psk[:, 0, :],
                    lhsT=w_k[:],
                    rhs=h_sb[:, j * 512 : (j + 1) * 512],
                    start=True,
                    stop=True,
                )
                nc.vector.tensor_copy(out=k_sb[:, j * 512 : (j + 1) * 512], in_=psk[:, 0, :])
            vT_sb = big.tile([128, MT, C], mmdt, tag="vt")
            for mi in range(MT):
                psv = ps_s.tile([C, EXP_GRP, 512], f32, tag="s")
                nc.tensor.matmul(
                    psv[:, 0, :C],
                    lhsT=h_sb[:, mi * 128 : (mi + 1) * 128].bitcast(f32),
                    rhs=w_v[:],
                    start=True,
                    stop=True,
                )
                nc.vector.tensor_copy(out=vT_sb[:, mi, :], in_=psv[:, 0, :C])

            # --- attention over query blocks ---
            for jb in range(NBLK):
                qs = q_sb[:, jb * NB : (jb + 1) * NB]
                pv = ps_pv.tile([C, NB], f32, tag="pv")
                dn = ps_den.tile([1, NB], f32, tag="dn")
                for g in range(MT // EXP_GRP):
                    ss = ps_s.tile([128, EXP_GRP, NB], f32, tag="s")
                    for u in range(EXP_GRP):
                        mi = g * EXP_GRP + u
                        nc.tensor.matmul(
                            ss[:, u, :],
                            lhsT=k_sb[:, mi * 128 : (mi + 1) * 128],
                            rhs=qs,
                            start=True,
                            stop=True,
                        )
                    pt = ptile.tile([128, EXP_GRP, NB], mmdt, tag="pt")
                    nc.scalar.activation(out=pt[:], in_=ss[:], func=Exp, scale=SCALE)
                    for u in range(EXP_GRP):
                        mi = g * EXP_GRP + u
                        nc.tensor.matmul(
                            pv[:],
                            lhsT=vT_sb[:, mi, :],
                            rhs=pt[:, u, :],
                            start=(mi == 0),
                            stop=(mi == MT - 1),
                        )
                    for u in range(EXP_GRP):
                        mi = g * EXP_GRP + u
                        nc.tensor.matmul(
                            dn[:],
                            lhsT=ones_sb[:],
                            rhs=pt[:, u, :],
                            start=(mi == 0),
                            stop=(mi == MT - 1),
                        )
                # denominator: reciprocal + partition broadcast
                dsb = small.tile([1, NB], f32, tag="dsb")
                nc.vector.tensor_copy(out=dsb[:], in_=dn[:])
                rden = small.tile([1, NB], f32, tag="rden")
                nc.vector.reciprocal_approx_fast(out=rden[:], in_=dsb[:])
                rb = ostage.tile([128, NB], f32, tag="rb")
                nc.gpsimd.partition_broadcast(rb[:], rden[:])
                # output projection on the un-normalized numerator
                hv = ostage.tile([C, NB], mmdt, tag="hv")
                nc.vector.tensor_copy(out=hv[:], in_=pv[:])
                pso = ps_m.tile([C, NB], f32, tag="m")
                nc.tensor.matmul(
                    pso[:], lhsT=w_p[:], rhs=hv[:], start=True, stop=True
                )
                o1 = ostage.tile([C, NB], f32, tag="o1")
                nc.vector.tensor_tensor(o1[:], pso[:], rb[:], mult_op)
                nc.vector.tensor_scalar_add(o1[:], o1[:], bp_sb[:])
                nc.vector.tensor_tensor(
                    o1[:], o1[:], x_sb[:, jb * NB : (jb + 1) * NB], add_op
                )
                nc.sync.dma_start(out=out_d[:, jb * NB : (jb + 1) * NB], in_=o1[:])

    nc.compile()
    _NC_CACHE[mm_fast] = nc
    return nc


def kernel(**inputs):
    global LAST_RESULTS
    _install_ntff_hook()
    from concourse.bass_utils import run_bass_kernel_spmd

    ins = {k: np.ascontiguousarray(np.asarray(v), dtype=np.float32) for k, v in inputs.items()}
    x = ins["x"]
    gs, gb = ins["gn_scale"], ins["gn_bias"]

    # Fold the GroupNorm affine into the q/k/v weights; pre-transpose all
    # weights into the [in_channel, out_channel] layout the PE wants.
    wq_e = ins["wq"] * gs[None, :]
    wk_e = ins["wk"] * gs[None, :]
    wv_e = ins["wv"] * gs[None, :]
    wqT = np.ascontiguousarray(wq_e.T)
    wkT = np.ascontiguousarray(wk_e.T)
    wvT = np.ascontiguousarray(wv_e.T)
    wpT = np.ascontiguousarray(ins["wp"].T)
    bq_e = (ins["bq"] + ins["wq"] @ gb).reshape(C, 1)
    bv_e = ins["bv"] + ins["wv"] @ gb
    bp_e = (ins["bp"] + ins["wp"] @ bv_e).reshape(C, 1)

    nc = _build(MM_FAST)

    in_maps = []
    for core in range(8):
        b, half = core // 2, core % 2
        xb = x[b].reshape(C, N)
        if half == 1:
            xb = np.concatenate([xb[:, NQ:], xb[:, :NQ]], axis=1)
        in_maps.append(
            {
                "xp": np.ascontiguousarray(xb),
                "wqT": wqT,
                "wkT": wkT,
                "wvT": wvT,
                "wpT": wpT,
                "bqe": bq_e,
                "bpe": bp_e,
            }
        )

    trace = os.environ.get("KERNEL_TRACE", "0") == "1"
    res = run_bass_kernel_spmd(
        nc, in_maps, core_ids=list(range(8)), trace=trace
    )
    LAST_RESULTS = res

    out = np.empty((B, C, N), np.float32)
    for core in range(8):
        b, half = core // 2, core % 2
        out[b, :, half * NQ : (half + 1) * NQ] = res.results[core]["out"]
    return out.reshape(B, C, H, W)


# revision 6
# speedup vs baseline: 1.1716x; 1.1386x over previous
"""AttnBlock (GroupNorm + single-head HWxHW attention + residual) on 8 trn2 cores.

Sharding: data-parallel over (batch, query-half): core i handles batch i//2,
query columns [ (i%2)*2048, (i%2+1)*2048 ).  The input for odd cores is
column-rotated on the host so every core's queries are columns 0:2048 of its
input (softmax over keys is permutation invariant, so k/v order doesn't
matter) -- this keeps the program SPMD (one NEFF for all 8 cores).

Device algorithm (per core, C=128 channels on partitions, N=4096 spatial):
  - GroupNorm stats: per-channel bn_stats/bn_aggr, then group (16-channel)
    reduction + broadcast via tiny mask matmuls on the PE.
  - q/k projections as [C,C]x[C,n] matmuls; v is produced directly
    TRANSPOSED (vT[m,c]) by using h-chunks as the stationary operand.
  - Scores are computed transposed: sT[m-tile, n-block] = k_tile^T . q_blk.
    exp() runs on the scalar engine straight out of PSUM over two banks at a
    time (no row-max subtraction needed: scores are O(10), fp32 exp is safe).
  - Softmax denominator (a cross-partition sum of the exp tiles) is split
    between engines to balance load: half the key tiles accumulate on the
    vector engine in SBUF, half accumulate on the PE into a PSUM row via
    ones-vector matmuls; the SBUF part is folded in with one more matmul.
  - PV: num[c, n] += vT_tile^T . pT_tile accumulated over 32 m-tiles in PSUM.
  - 1/den (fast Newton reciprocal on DVE) is broadcast across partitions by
    GPSIMD and fused into the PV-drain copy; the output projection then only
    needs a single residual add: out = x + wp.(num/den) [+ bias].

Host folding: gn_scale/gn_bias are folded into the q/k/v weights and biases;
the k bias is dropped entirely (additive per-query constant is softmax
invariant); the v bias is folded into the output projection bias since
softmax rows sum to 1.  Remaining biases are per-partition scalar adds, only
emitted when nonzero.

Matmuls run in float32r (fast fp32 mode, 1 PE cycle/row); all tiles feeding
them are produced pre-rounded (hardware requirement).
"""

import os
import sys
import types

if "/opt/trn_rl_repo" not in sys.path:
    sys.path.insert(0, "/opt/trn_rl_repo")

import numpy as np

B, C, H, W = 4, 128, 64, 64
N = H * W              # 4096 spatial positions
NQ = N // 2            # 2048 queries per core
NB = 512               # query block (columns per psum bank)
NBLK = NQ // NB        # 4 query blocks
MT = N // 128          # 32 key tiles
NCH = 4                # x/h chunking (1024 columns per chunk)
GROUPS = 8
GSIZE = C // GROUPS    # 16 channels per group
EPS = 1e-6
SCALE = float(C) ** -0.5
EXP_GRP = 2            # psum banks (512-wide matmuls) per exp instruction
DVE_DEN_G = 8          # exp groups whose denominator accumulates on DVE (of 16)

# Set to False to run all big matmuls in full fp32 (4x slower, exact).
MM_FAST = os.environ.get("KERNEL_MM_FAST", "1") == "1"

LAST_RESULTS = None    # BassKernelResults of the most recent kernel() call


def _install_ntff_hook():
    """antenv.axon_hooks is missing from this container; inject it so
    run_bass_kernel_spmd(trace=True) can capture NTFF profiles."""
    if "antenv.axon_hooks" in sys.modules:
        return
    mod = types.ModuleType("antenv.axon_hooks")
    holder = [None]
    mod.set_axon_ntff_profile_hook = lambda h: holder.__setitem__(0, h)
    mod.get_axon_ntff_profile_hook = lambda: holder[0]
    sys.modules["antenv.axon_hooks"] = mod
    try:
        from trn_agent_boot.trn_boot import _ntff_profile_via_ctypes

        mod.set_axon_ntff_profile_hook(
            _ntff_profile_via_ctypes("/opt/axon/libaxon_pjrt.so")
        )
    except Exception:
        pass


_NC_CACHE = {}


def _build(mm_fast: bool, use_bq: bool, use_bp: bool):
    key = (mm_fast, use_bq, use_bp)
    if key in _NC_CACHE:
        return _NC_CACHE[key]

    import concourse.bacc as bacc
    import concourse.mybir as mybir
    import concourse.tile as tile

    f32 = mybir.dt.float32
    mmdt = mybir.dt.float32r if mm_fast else f32

    nc = bacc.Bacc("TRN2", target_bir_lowering=False, debug=False, num_devices=8)

    xp = nc.dram_tensor("xp", [C, N], f32, kind="ExternalInput")
    wqT_d = nc.dram_tensor("wqT", [C, C], f32, kind="ExternalInput")
    wkT_d = nc.dram_tensor("wkT", [C, C], f32, kind="ExternalInput")
    wvT_d = nc.dram_tensor("wvT", [C, C], f32, kind="ExternalInput")
    wpT_d = nc.dram_tensor("wpT", [C, C], f32, kind="ExternalInput")
    bq_d = nc.dram_tensor("bqe", [C, 1], f32, kind="ExternalInput")
    bp_d = nc.dram_tensor("bpe", [C, 1], f32, kind="ExternalInput")
    out_d = nc.dram_tensor("out", [C, NQ], f32, kind="ExternalOutput")

    # Group-mean reduction masks: gm averages a group's 16 channels (and the
    # spatial dim) into one row; gmT broadcasts group rows back to channels.
    gm_np = np.zeros((C, GROUPS), np.float32)
    gmT_np = np.zeros((GROUPS, C), np.float32)
    for ch in range(C):
        gm_np[ch, ch // GSIZE] = 1.0 / (GSIZE * N)
        gmT_np[ch // GSIZE, ch] = 1.0
    gm_d = nc.inline_tensor(gm_np, "gmask")
    gmT_d = nc.inline_tensor(gmT_np, "gmaskT")

    Exp = mybir.ActivationFunctionType.Exp
    Sqrt = mybir.ActivationFunctionType.Sqrt
    add_op = mybir.AluOpType.add
    sub_op = mybir.AluOpType.subtract
    mult_op = mybir.AluOpType.mult
    CHW = N // NCH  # 1024

    with tile.TileContext(nc) as tc:
        with (
            tc.tile_pool(name="big", bufs=1) as big,
            tc.tile_pool(name="wgt", bufs=1) as wgt,
            tc.tile_pool(name="ptile", bufs=6) as ptile,
            tc.tile_pool(name="small", bufs=2) as small,
            tc.tile_pool(name="ostage", bufs=2) as ostage,
            tc.tile_pool(name="ps_s", bufs=2, space="PSUM") as ps_s,
            tc.tile_pool(name="ps_pv", bufs=2, space="PSUM") as ps_pv,
            tc.tile_pool(name="ps_den", bufs=1, space="PSUM") as ps_den,
            tc.tile_pool(name="ps_m", bufs=1, space="PSUM") as ps_m,
        ):
            # --- load inputs (x in chunks so stats overlap the DMA) ---
            xc = []
            for j in range(NCH):
                xj = big.tile([C, CHW], f32, tag=f"x{j}")
                nc.sync.dma_start(out=xj[:], in_=xp.ap()[:, j * CHW : (j + 1) * CHW])
                xc.append(xj)
            w_q0 = wgt.tile([C, C], f32, tag="wq0")
            nc.sync.dma_start(out=w_q0[:], in_=wqT_d.ap())
            w_k0 = wgt.tile([C, C], f32, tag="wk0")
            nc.sync.dma_start(out=w_k0[:], in_=wkT_d.ap())
            w_v = wgt.tile([C, C], f32, tag="wv")
            nc.sync.dma_start(out=w_v[:], in_=wvT_d.ap())
            w_p0 = wgt.tile([C, C], f32, tag="wp0")
            nc.sync.dma_start(out=w_p0[:], in_=wpT_d.ap())
            if mm_fast:
                w_q = wgt.tile([C, C], mmdt, tag="wq")
                nc.vector.tensor_copy(out=w_q[:], in_=w_q0[:])
                w_k = wgt.tile([C, C], mmdt, tag="wk")
                nc.vector.tensor_copy(out=w_k[:], in_=w_k0[:])
                w_p = wgt.tile([C, C], mmdt, tag="wp")
                nc.vector.tensor_copy(out=w_p[:], in_=w_p0[:])
            else:
                w_q, w_k, w_p = w_q0, w_k0, w_p0
            if use_bq:
                bq_sb = wgt.tile([C, 1], f32, tag="bq")
                nc.sync.dma_start(out=bq_sb[:], in_=bq_d.ap())
            if use_bp:
                bp_sb = wgt.tile([C, 1], f32, tag="bp")
                nc.sync.dma_start(out=bp_sb[:], in_=bp_d.ap())
            gm_sb = wgt.tile([C, GROUPS], f32, tag="gm")
            nc.sync.dma_start(out=gm_sb[:], in_=gm_d.ap())
            gmT_sb = wgt.tile([GROUPS, C], f32, tag="gmT")
            nc.sync.dma_start(out=gmT_sb[:], in_=gmT_d.ap())
            ones0 = wgt.tile([C, 1], f32, tag="ones0")
            nc.vector.memset(ones0[:], 1.0)
            if mm_fast:
                ones_sb = wgt.tile([C, 1], mmdt, tag="ones")
                nc.vector.tensor_copy(out=ones_sb[:], in_=ones0[:])
            else:
                ones_sb = ones0
            eps_sb = wgt.tile([C, 1], f32, tag="eps")
            nc.vector.memset(eps_sb[:], EPS)

            # --- GroupNorm statistics ---
            stats = small.tile([C, 8, 6], f32, tag="stats")
            for j in range(8):
                nc.vector.bn_stats(
                    out=stats[:, j, :],
                    in_=xc[j // 2][:, (j % 2) * 512 : (j % 2) * 512 + 512],
                )
            mv = small.tile([C, 2], f32, tag="mv")
            nc.vector.bn_aggr(out=mv[:], in_=stats[:])
            # t2 = per-channel [sum(x), sum(x^2)]; gm then averages over the group
            t2 = small.tile([C, 2], f32, tag="t2")
            nc.vector.tensor_scalar_mul(t2[:, 0:1], mv[:, 0:1], float(N))
            nc.vector.tensor_tensor(t2[:, 1:2], mv[:, 0:1], mv[:, 0:1], mult_op)
            nc.vector.tensor_tensor(t2[:, 1:2], t2[:, 1:2], mv[:, 1:2], add_op)
            nc.vector.tensor_scalar_mul(t2[:, 1:2], t2[:, 1:2], float(N))
            psg = ps_m.tile([GROUPS, 2], f32, tag="m")
            nc.tensor.matmul(psg[:], lhsT=gm_sb[:], rhs=t2[:], start=True, stop=True)
            g2 = small.tile([GROUPS, 2], f32, tag="g2")
            nc.vector.tensor_copy(out=g2[:], in_=psg[:])
            psb = ps_m.tile([C, 2], f32, tag="m")
            nc.tensor.matmul(psb[:], lhsT=gmT_sb[:], rhs=g2[:], start=True, stop=True)
            # mu = E[x]; var = E[x^2] - mu^2 ; rstd = 1/sqrt(var+eps)
            mu = small.tile([C, 1], f32, tag="mu")
            nc.vector.tensor_copy(out=mu[:], in_=psb[:, 0:1])
            var = small.tile([C, 1], f32, tag="var")
            nc.vector.tensor_tensor(var[:], mu[:], mu[:], mult_op)
            nc.vector.tensor_tensor(var[:], psb[:, 1:2], var[:], sub_op)
            sd = small.tile([C, 1], f32, tag="sd")
            nc.scalar.activation(out=sd[:], in_=var[:], func=Sqrt, bias=eps_sb[:])
            rstd = small.tile([C, 1], f32, tag="rstd")
            nc.vector.reciprocal_approx_fast(out=rstd[:], in_=sd[:])

            # h = (x - mu) * rstd, chunked so downstream matmuls start early
            hc = []
            for j in range(NCH):
                hj = big.tile([C, CHW], mmdt, tag=f"h{j}")
                nc.vector.tensor_scalar(
                    hj[:], xc[j][:], mu[:], rstd[:], op0=sub_op, op1=mult_op
                )
                hc.append(hj)

            def hpart(lo, width):
                j = lo // CHW
                assert lo + width <= (j + 1) * CHW
                return hc[j][:, lo - j * CHW : lo - j * CHW + width]

            # --- q, k, vT projections ---
            qb = []
            for j in range(NBLK):
                psq = ps_s.tile([C, EXP_GRP, 512], f32, tag="s")
                nc.tensor.matmul(
                    psq[:, 0, :],
                    lhsT=w_q[:],
                    rhs=hpart(j * 512, 512),
                    start=True,
                    stop=True,
                )
                qj = big.tile([C, NB], mmdt, tag=f"q{j}")
                if use_bq:
                    nc.vector.tensor_scalar_add(qj[:], psq[:, 0, :], bq_sb[:])
                else:
                    nc.vector.tensor_copy(out=qj[:], in_=psq[:, 0, :])
                qb.append(qj)
            kc = []
            for j in range(8):
                psk = ps_s.tile([C, EXP_GRP, 512], f32, tag="s")
                nc.tensor.matmul(
                    psk[:, 0, :],
                    lhsT=w_k[:],
                    rhs=hpart(j * 512, 512),
                    start=True,
                    stop=True,
                )
                kj = big.tile([C, 512], mmdt, tag=f"k{j}")
                nc.vector.tensor_copy(out=kj[:], in_=psk[:, 0, :])
                kc.append(kj)

            def kpart(mi):
                return kc[mi // 4][:, (mi % 4) * 128 : (mi % 4) * 128 + 128]

            vT_sb = big.tile([128, MT, C], mmdt, tag="vt")
            for mi in range(MT):
                psv = ps_s.tile([C, EXP_GRP, 512], f32, tag="s")
                nc.tensor.matmul(
                    psv[:, 0, :C],
                    lhsT=hpart(mi * 128, 128).bitcast(f32),
                    rhs=w_v[:],
                    start=True,
                    stop=True,
                )
                nc.vector.tensor_copy(out=vT_sb[:, mi, :], in_=psv[:, 0, :C])

            # --- attention over query blocks ---
            for jb in range(NBLK):
                qs = qb[jb][:]
                pv = ps_pv.tile([C, NB], f32, tag="pv")
                dn = ps_den.tile([1, NB], f32, tag="dn")
                dacc = ostage.tile([128, EXP_GRP, NB], f32, tag="dacc")
                for g in range(MT // EXP_GRP):
                    ss = ps_s.tile([128, EXP_GRP, NB], f32, tag="s")
                    for u in range(EXP_GRP):
                        mi = g * EXP_GRP + u
                        nc.tensor.matmul(
                            ss[:, u, :],
                            lhsT=kpart(mi),
                            rhs=qs,
                            start=True,
                            stop=True,
                        )
                    pt = ptile.tile([128, EXP_GRP, NB], mmdt, tag="pt")
                    nc.scalar.activation(out=pt[:], in_=ss[:], func=Exp, scale=SCALE)
                    for u in range(EXP_GRP):
                        mi = g * EXP_GRP + u
                        nc.tensor.matmul(
                            pv[:],
                            lhsT=vT_sb[:, mi, :],
                            rhs=pt[:, u, :],
                            start=(mi == 0),
                            stop=(mi == MT - 1),
                        )
                    if g < DVE_DEN_G:
                        # denominator partial on DVE (SBUF adds)
                        ptf = pt.bitcast(f32)
                        if g == 0:
                            nc.vector.tensor_copy(out=dacc[:], in_=ptf[:])
                        else:
                            nc.vector.tensor_tensor(dacc[:], dacc[:], ptf[:], add_op)
                    else:
                        # denominator partial on PE (cross-partition sum)
                        for u in range(EXP_GRP):
                            mi = g * EXP_GRP + u
                            nc.tensor.matmul(
                                dn[:],
                                lhsT=ones_sb[:],
                                rhs=pt[:, u, :],
                                start=(g == DVE_DEN_G and u == 0),
                                stop=False,
                            )
                # fold the DVE partial into the PSUM denominator row
                dfold = ostage.tile([128, NB], mmdt, tag="dfold")
                nc.vector.tensor_tensor(dfold[:], dacc[:, 0, :], dacc[:, 1, :], add_op)
                nc.tensor.matmul(
                    dn[:], lhsT=ones_sb[:], rhs=dfold[:], start=False, stop=True
                )
                dsb = small.tile([1, NB], f32, tag="dsb")
                nc.vector.tensor_copy(out=dsb[:], in_=dn[:])
                rden = small.tile([1, NB], f32, tag="rden")
                nc.vector.reciprocal_approx_fast(out=rden[:], in_=dsb[:])
                rb = ostage.tile([128, NB], f32, tag="rb")
                nc.gpsimd.partition_broadcast(rb[:], rden[:])
                # normalize during the PV drain, then project and add residual
                hv = ostage.tile([C, NB], mmdt, tag="hv")
                nc.vector.tensor_tensor(hv[:], pv[:], rb[:], mult_op)
                pso = ps_m.tile([C, NB], f32, tag="m")
                nc.tensor.matmul(
                    pso[:], lhsT=w_p[:], rhs=hv[:], start=True, stop=True
                )
                o1 = ostage.tile([C, NB], f32, tag="o1")
                xblk = xc[jb // 2][:, (jb % 2) * 512 : (jb % 2) * 512 + 512]
                nc.vector.tensor_tensor(o1[:], pso[:], xblk, add_op)
                if use_bp:
                    nc.vector.tensor_scalar_add(o1[:], o1[:], bp_sb[:])
                nc.sync.dma_start(out=out_d[:, jb * NB : (jb + 1) * NB], in_=o1[:])

    nc.compile()
    _NC_CACHE[key] = nc
    return nc


def kernel(**inputs):
    global LAST_RESULTS
    _install_ntff_hook()
    from concourse.bass_utils import run_bass_kernel_spmd

    ins = {
        k: np.ascontiguousarray(np.asarray(v), dtype=np.float32)
        for k, v in inputs.items()
    }
    x = ins["x"]
    gs, gb = ins["gn_scale"], ins["gn_bias"]

    # Fold the GroupNorm affine into the q/k/v weights; pre-transpose all
    # weights into the [in_channel, out_channel] layout the PE wants.
    wq_e = ins["wq"] * gs[None, :]
    wk_e = ins["wk"] * gs[None, :]
    wv_e = ins["wv"] * gs[None, :]
    wqT = np.ascontiguousarray(wq_e.T)
    wkT = np.ascontiguousarray(wk_e.T)
    wvT = np.ascontiguousarray(wv_e.T)
    wpT = np.ascontiguousarray(ins["wp"].T)
    bq_e = (ins["bq"] + ins["wq"] @ gb).reshape(C, 1)
    bv_e = ins["bv"] + ins["wv"] @ gb
    bp_e = (ins["bp"] + ins["wp"] @ bv_e).reshape(C, 1)
    use_bq = bool(np.any(bq_e))
    use_bp = bool(np.any(bp_e))

    nc = _build(MM_FAST, use_bq, use_bp)

    in_maps = []
    for core in range(8):
        b, half = core // 2, core % 2
        xb = x[b].reshape(C, N)
        if half == 1:
            xb = np.concatenate([xb[:, NQ:], xb[:, :NQ]], axis=1)
        in_maps.append(
            {
                "xp": np.ascontiguousarray(xb),
                "wqT": wqT,
                "wkT": wkT,
                "wvT": wvT,
                "wpT": wpT,
                "bqe": bq_e,
                "bpe": bp_e,
            }
        )

    trace = os.environ.get("KERNEL_TRACE", "0") == "1"
    res = run_bass_kernel_spmd(nc, in_maps, core_ids=list(range(8)), trace=trace)
    LAST_RESULTS = res

    out = np.empty((B, C, N), np.float32)
    for core in range(8):
        b, half = core // 2, core % 2
        out[b, :, half * NQ : (half + 1) * NQ] = res.results[core]["out"]
    return out.reshape(B, C, H, W)
